# revision 43
# baseline (speedup 1.0000x reference)
"""Causal multi-head self-attention (RoPE on input) for Trainium2, 8 NeuronCores.

Sharding: core c handles batch b = c//2 and head-group g = c%2 (8 of 16 heads).
Wq/Wk/Wv are split column-wise per head-group, Wo row-wise; each core produces a
partial (T, E) output and the host sums the two head-group partials per batch
and adds the bias.

v2 design notes (vs the phase-separated v1):
- All host-side tensors are laid out so every DMA is contiguous per
  partition line (no on-the-fly rearranges -> ~10x fewer descriptors).
- RoPE runs in place on the xt tile (products into temps first, then the
  pair-combine writes back), saving a 32KB/partition rx copy.
- The whole kernel is one software pipeline over query blocks i (512
  tokens): per (i, hp) the Q/K projection for token block i is emitted,
  then scores+exp+mask with PV trailing by 2 key tiles; the output
  projection for block i-1 is interleaved into block i so there is no
  serial phase C tail.
- Diagonal score/PV matmuls stream only the causally-live query suffix,
  and the mask multiply covers only the 128-wide triangle column block.
- Softmax normalization: denominators come from a ones column in V; a
  fast approximate reciprocal per pair is broadcast across partitions
  via a DRAM bounce, staggered two pairs behind the attention pipeline.
"""

import numpy as np
import ml_dtypes

import concourse.bacc as bacc
import concourse.tile as tile
import concourse.mybir as mybir
from concourse import bass_utils
from concourse.bass_interp import get_hw_module

bf16 = ml_dtypes.bfloat16
BF = mybir.dt.bfloat16
F32 = mybir.dt.float32
EXP = mybir.ActivationFunctionType.Exp

B, T, E = 4, 2048, 1024
H, HD = 16, 64
G = 2  # head groups (tensor-parallel dimension)
HL = H // G  # heads per core
DL = HL * HD  # 512 local feature dim
P = 128
NT = T // P  # 16 token tiles
NQ = T // 512  # 4 query blocks
EC = E // P  # 8 contraction chunks over E
DC = DL // P  # 4 chunks over local head dims (one per head pair)

_CACHE = {}
LAST_RESULT = None


def _build():
    nc = bacc.Bacc("TRN2", target_bir_lowering=False, debug=False, num_devices=8)
    xt_d = nc.dram_tensor("xt", (NT, P, EC, P), BF, kind="ExternalInput").ap()
    sin_d = nc.dram_tensor("sin", (P, 2, 4, 1024), BF, kind="ExternalInput").ap()
    cos_d = nc.dram_tensor("cos", (P, 2, 4, 1024), BF, kind="ExternalInput").ap()
    wq_d = nc.dram_tensor("wq", (P, EC, DL), BF, kind="ExternalInput").ap()
    wk_d = nc.dram_tensor("wk", (P, EC, DL), BF, kind="ExternalInput").ap()
    wv_d = nc.dram_tensor("wv", (P, EC, DL), BF, kind="ExternalInput").ap()
    wo_d = nc.dram_tensor("wo", (P, DC, E), BF, kind="ExternalInput").ap()
    mask_d = nc.dram_tensor("mask", (P, P), BF, kind="ExternalInput").ap()
    out_d = nc.dram_tensor("out", (T, E), BF, kind="ExternalOutput").ap()

    with tile.TileContext(nc) as tc:
        with (
            tc.tile_pool(name="persist", bufs=1) as persist,
            tc.tile_pool(name="tabs", bufs=1) as tabs,
            tc.tile_pool(name="tmps", bufs=1) as tmps,
            tc.tile_pool(name="att", bufs=6) as apool,
            tc.tile_pool(name="pks", bufs=2) as pkp,
            tc.tile_pool(name="rbs", bufs=3) as rbp,
            tc.tile_pool(name="ots", bufs=4) as otp,
            tc.tile_pool(name="dramn", bufs=3, space="DRAM") as dpool,
            tc.tile_pool(name="ps512", bufs=2, space="PSUM") as ps512,
            tc.tile_pool(name="sps", bufs=2, space="PSUM") as spool,
            tc.tile_pool(name="ops", bufs=2, space="PSUM") as opool,
        ):
            xt = persist.tile([P, NT, EC, P], BF)
            qT = persist.tile([P, DC, T], BF)
            kT = persist.tile([P, DC, T], BF)
            v = persist.tile([P, NT, HL, HD + 1], BF)
            oc = persist.tile([P, DC, T], BF)
            wq = persist.tile([P, EC, DL], BF)
            wk = persist.tile([P, EC, DL], BF)
            wv = persist.tile([P, EC, DL], BF)
            wo = persist.tile([P, DC, E], BF)
            mask = persist.tile([P, P], BF)
            sel = persist.tile([1, HD], BF)

            sins = [tabs.tile([P, 4, 1024], BF, name=f"s{h}") for h in range(2)]
            coss = [tabs.tile([P, 4, 1024], BF, name=f"c{h}") for h in range(2)]

            # ---- DMA emission = approximate arrival order; sequenced so
            # each consumer unblocks as early as possible: V-proj needs
            # wv+xt[k]; RoPE chunk 0 needs xt[0:4] + the per-u table slices;
            # the first K-proj needs wk + RoPE chunk 0.
            # Spread DMA issue over the two HWDGE queues (each issue costs
            # ~650ns serial on its queue; gpsimd/SWDGE generates descriptors
            # in ucode and is far too slow for bulk transfers).
            nc.sync.dma_start(wv[:, 0:2, :], wv_d[:, 0:2])
            nc.sync.dma_start(xt[:, 0:1], xt_d[0:1].rearrange("t p e c -> p t e c"))
            nc.sync.dma_start(wv[:, 2:4, :], wv_d[:, 2:4])
            nc.scalar.dma_start(sins[0][:, 2, :], sin_d[:, 0, 2])
            nc.scalar.dma_start(coss[0][:, 2, :], cos_d[:, 0, 2])
            nc.sync.dma_start(wv[:, 4:8, :], wv_d[:, 4:8])
            nc.scalar.dma_start(sins[0][:, 3, :], sin_d[:, 0, 3])
            nc.scalar.dma_start(coss[0][:, 3, :], cos_d[:, 0, 3])
            nc.sync.dma_start(sins[0][:, 0, :], sin_d[:, 0, 0])
            nc.sync.dma_start(coss[0][:, 0, :], cos_d[:, 0, 0])
            nc.scalar.dma_start(sins[0][:, 1, :], sin_d[:, 0, 1])
            nc.scalar.dma_start(coss[0][:, 1, :], cos_d[:, 0, 1])
            nc.sync.dma_start(xt[:, 1:2], xt_d[1:2].rearrange("t p e c -> p t e c"))
            nc.sync.dma_start(xt[:, 2:4], xt_d[2:4].rearrange("t p e c -> p t e c"))
            nc.scalar.dma_start(wq, wq_d)
            nc.sync.dma_start(xt[:, 4:6], xt_d[4:6].rearrange("t p e c -> p t e c"))
            nc.sync.dma_start(xt[:, 6:8], xt_d[6:8].rearrange("t p e c -> p t e c"))

            nc.vector.memset(v[:, :, :, HD : HD + 1], 1.0)
            nc.vector.memset(sel, 1.0)

            def emit_vproj(tk):
                vp = ps512.tile([P, DL], F32, tag="ps512")
                for j in range(EC):
                    nc.tensor.matmul(
                        vp,
                        lhsT=xt[:, tk, j, :],
                        rhs=wv[:, j, :],
                        start=(j == 0),
                        stop=(j == EC - 1),
                    )
                nc.scalar.copy(
                    v[:, tk, :, 0:HD], vp.rearrange("p (h d) -> p h d", h=HL)
                )

            def emit_rope(tc_):
                """RoPE (in place) for token chunk tc_ (512 tokens). All on
                DVE: offloading u-chunks to GpSimd was tested and reverted --
                concurrent engines on adjacent xt slices contend for SBUF
                ports and slow both to ~3x per-op time."""
                half, qh = tc_ // 2, tc_ % 2
                ts = slice(4 * tc_, 4 * tc_ + 4)
                cs = slice(512 * qh, 512 * qh + 512)
                for u in range(4):
                    # 4 DVE ops per u instead of 6: multiply the (even, odd)
                    # lane pair by cos and sin in one strided op each, then
                    # combine.  xp = xt[:, ts, u::4-stride over (u, u+4)]
                    xe = xt[:, ts, u, :]
                    xo = xt[:, ts, u + 4, :]
                    xp = xt[:, ts, u : u + 5 : 4, :]
                    s_u = sins[half][:, u, cs].rearrange("p (a b) -> p a b", a=4)
                    c_u = coss[half][:, u, cs].rearrange("p (a b) -> p a b", a=4)
                    s_b = s_u[:, :, None, :].to_broadcast((P, 4, 2, P))
                    c_b = c_u[:, :, None, :].to_broadcast((P, 4, 2, P))
                    tc2 = tmps.tile([P, 4, 2, P], BF, tag="tc2")
                    ts2 = tmps.tile([P, 4, 2, P], BF, tag="ts2")
                    nc.vector.tensor_mul(tc2, xp, c_b)
                    nc.vector.tensor_mul(ts2, xp, s_b)
                    nc.vector.tensor_sub(xe, tc2[:, :, 0, :], ts2[:, :, 1, :])
                    nc.vector.tensor_add(xo, tc2[:, :, 1, :], ts2[:, :, 0, :])

            def emit_qk(hp, i):
                """Q+K projection for token block i of pair hp."""
                tq = slice(512 * i, 512 * (i + 1))
                # Q first: scores consume fresh qT at j=0, but the fresh kT
                # tiles only at the diagonal (end of the j loop), so the Q
                # copy hides under the K projection matmuls.
                for w_sb, dst in ((wq, qT), (wk, kT)):
                    pp = ps512.tile([P, 512], F32, tag="ps512")
                    for j in range(EC):
                        nc.tensor.matmul(
                            pp,
                            lhsT=w_sb[:, j, P * hp : P * (hp + 1)],
                            rhs=xt[:, 4 * i : 4 * i + 4, j, :],
                            start=(j == 0),
                            stop=(j == EC - 1),
                        )
                    nc.vector.tensor_copy(dst[:, hp, tq], pp)

            def emit_outproj_groups(i, tsub, alt=False):
                """Output projection for token sub-tile tsub (0..3) of block i."""
                tt = 4 * i + tsub
                for et in range(2):
                    pp = ps512.tile([P, 512], F32, tag="ps512")
                    for kk in range(DC):
                        nc.tensor.matmul(
                            pp,
                            lhsT=oc[:, kk, P * tt : P * (tt + 1)],
                            rhs=wo[:, kk, 512 * et : 512 * (et + 1)],
                            start=(kk == 0),
                            stop=(kk == DC - 1),
                        )
                    ot = otp.tile([P, 512], BF, tag="ot")
                    if alt and et == 0:
                        nc.scalar.copy(ot, pp)
                    else:
                        nc.vector.tensor_copy(ot, pp)
                    eng = nc.scalar if (alt and et == 1) else nc.sync
                    eng.dma_start(
                        out_d[P * tt : P * (tt + 1), 512 * et : 512 * (et + 1)], ot
                    )

            def emit_attn(hp, i):
                """Scores+exp+mask with trailing PV for (block i, pair hp)."""
                h0, h1 = 2 * hp, 2 * hp + 1
                nj = 4 * i + 4
                tq0 = 512 * i
                op0 = opool.tile([HD + 1, 512], F32, tag="o")
                op1 = opool.tile([HD + 1, 512], F32, tag="o")
                ats = []

                def emit_pv(jp, at_jp):
                    lo = max(0, P * (jp - 4 * i))
                    nc.tensor.matmul(
                        op0[:, lo:512],
                        lhsT=v[:, jp, h0, :],
                        rhs=at_jp[:, 0, lo:512],
                        start=(jp == 0),
                        stop=(jp == nj - 1),
                    )
                    nc.tensor.matmul(
                        op1[:, lo:512],
                        lhsT=v[:, jp, h1, :],
                        rhs=at_jp[:, 1, lo:512],
                        start=(jp == 0),
                        stop=(jp == nj - 1),
                    )

                for j in range(nj):
                    r = j - 4 * i
                    lo = max(0, P * r)
                    sp = spool.tile([P, 2, 512], F32, tag="s")
                    nc.tensor.matmul(
                        sp[:, 0, lo:512],
                        lhsT=kT[0:HD, hp, P * j : P * (j + 1)],
                        rhs=qT[0:HD, hp, tq0 + lo : tq0 + 512],
                        start=True,
                        stop=True,
                    )
                    nc.tensor.matmul(
                        sp[:, 1, lo:512],
                        lhsT=kT[HD:P, hp, P * j : P * (j + 1)],
                        rhs=qT[HD:P, hp, tq0 + lo : tq0 + 512],
                        start=True,
                        stop=True,
                    )
                    at = apool.tile([P, 2, 512], BF, tag="a")
                    nc.scalar.activation(
                        at[:, :, lo:512], sp[:, :, lo:512], EXP, scale=0.125
                    )
                    if r >= 0:
                        # only the 128-wide diagonal block needs the triangle
                        # mask; columns beyond it are fully live
                        nc.vector.tensor_mul(
                            at[:, :, lo : lo + P],
                            at[:, :, lo : lo + P],
                            mask[:, None, :].to_broadcast((P, 2, P)),
                        )
                    ats.append(at)
                    if j >= 2:
                        emit_pv(j - 2, ats[j - 2])
                for jp in range(max(nj - 2, 0), nj):
                    emit_pv(jp, ats[jp])

                # denominators (ones-column rows) -> staging at partition 0
                # (engine APs need aligned partition bases)
                s0 = pkp.tile([1, 512], F32, tag="sd", bufs=4)
                s1 = pkp.tile([1, 512], F32, tag="sd", bufs=4)
                nc.scalar.copy(s0, op0[HD : HD + 1, :])
                nc.scalar.copy(s1, op1[HD : HD + 1, :])
                # oc copies are deferred into finish_norm / finish_norm_fast
                # (one pair later) so the next pair's fresh Q copy goes first
                # on the DVE queue
                return s0, s1, op0, op1

            def finish_norm(i, hp, s0, s1, op0, op1):
                """Normalize oc for (i, hp): reciprocal rows broadcast across
                partitions via two K=1 matmuls (ones(1,64) x row) into one
                PSUM tile, then a single fused multiply into oc. Emitted one
                pair later so the PE never waits on the reciprocal chain."""
                tq = slice(512 * i, 512 * (i + 1))
                nc.vector.tensor_copy(oc[0:HD, hp, tq], op0[0:HD, :])
                nc.vector.tensor_copy(oc[HD:P, hp, tq], op1[0:HD, :])
                r0 = pkp.tile([1, 512], F32, tag="re", bufs=2)
                r1 = pkp.tile([1, 512], F32, tag="re", bufs=2)
                nc.vector.reciprocal_approx_fast(r0, s0)
                nc.vector.reciprocal_approx_fast(r1, s1)
                rd = dpool.tile([2, 512], F32, tag="rd", name=f"rd{i}_{hp}")
                nc.sync.dma_start(rd[0:1, :], r0)
                nc.sync.dma_start(rd[1:2, :], r1)
                rb = rbp.tile([P, 512], F32, tag="rb")
                nc.sync.dma_start(rb[0:HD, :], rd[0:1, :].to_broadcast((HD, 512)))
                nc.sync.dma_start(rb[HD:P, :], rd[1:2, :].to_broadcast((HD, 512)))
                return rb

            def finish_mul(i, hp, rb):
                tq = slice(512 * i, 512 * (i + 1))
                nc.vector.tensor_mul(oc[:, hp, tq], oc[:, hp, tq], rb)

            def finish_norm_fast(i, hp, s0, s1, op0, op1):
                """Tail-only normalize: broadcast via two K=1 PE matmuls
                (ones(1,64) x reciprocal row) instead of the DMA bounce --
                the PE is idle at the tail, and this chain is ~4us shorter."""
                tq = slice(512 * i, 512 * (i + 1))
                r0 = pkp.tile([1, 512], F32, tag="re", bufs=2)
                r1 = pkp.tile([1, 512], F32, tag="re", bufs=2)
                nc.vector.reciprocal_approx_fast(r0, s0)
                nc.vector.reciprocal_approx_fast(r1, s1)
                rc0 = pkp.tile([1, 512], BF, tag="rc", bufs=2)
                rc1 = pkp.tile([1, 512], BF, tag="rc", bufs=2)
                nc.vector.tensor_copy(rc0, r0)
                nc.vector.tensor_copy(rc1, r1)
                nc.scalar.copy(oc[0:HD, hp, tq], op0[0:HD, :])
                nc.scalar.copy(oc[HD:P, hp, tq], op1[0:HD, :])
                rb = ps512.tile([P, 512], F32, tag="ps512")
                nc.tensor.matmul(rb[0:HD, :], lhsT=sel, rhs=rc0, start=True, stop=True)
                nc.tensor.matmul(rb[HD:P, :], lhsT=sel, rhs=rc1, start=True, stop=True)
                nc.vector.tensor_mul(oc[:, hp, tq], oc[:, hp, tq], rb)

            # ---- pipeline ----
            for tk in range(6):
                emit_vproj(tk)
            nc.scalar.dma_start(mask, mask_d)
            emit_vproj(6)
            emit_vproj(7)
            # late-needed bytes (~6MB): issued from the scalar queue behind
            # the V-proj copies so their descriptors don't steal bandwidth
            # from the startup-critical set
            nc.scalar.dma_start(wk, wk_d)
            nc.scalar.dma_start(xt[:, 8:12], xt_d[8:12].rearrange("t p e c -> p t e c"))
            nc.scalar.dma_start(xt[:, 12:16], xt_d[12:16].rearrange("t p e c -> p t e c"))
            nc.scalar.dma_start(sins[1], sin_d[:, 1])
            nc.scalar.dma_start(coss[1], cos_d[:, 1])
            nc.scalar.dma_start(wo, wo_d)
            emit_rope(0)

            bounce_q = None  # (i, hp, s0, s1): reciprocal+broadcast next pair
            mul_q = None  # (i, hp, rb): normalize-multiply the pair after
            for i in range(NQ):
                for hp in range(DC):
                    emit_qk(hp, i)
                    if mul_q is not None:
                        finish_mul(*mul_q)
                        mul_q = None
                    if bounce_q is not None:
                        bi, bhp = bounce_q[0], bounce_q[1]
                        mul_q = (bi, bhp, finish_norm(*bounce_q))
                    bounce_q = (i, hp) + emit_attn(hp, i)
                    if i == 0:
                        emit_vproj(8 + 2 * hp)
                        emit_vproj(9 + 2 * hp)
                    if hp == 2 and i < 3:
                        emit_rope(i + 1)
                    if i > 0 and hp >= 1:
                        emit_outproj_groups(i - 1, hp - 1)
                        if hp == 3:
                            emit_outproj_groups(i - 1, 3)
            bi, bhp = bounce_q[0], bounce_q[1]
            finish_mul(*mul_q)
            finish_norm_fast(bi, bhp, *bounce_q[2:])
            for tsub in range(4):
                emit_outproj_groups(NQ - 1, tsub, alt=True)

    nc.compile()
    nc.m = get_hw_module(nc.m)
    return nc


def _prep_inputs(input, Wq, Wk, Wv, Wo):
    """Host-side shard prep: transpose/de-interleave/cast. Returns 8 in_maps."""
    perm = np.concatenate([np.arange(0, E, 2), np.arange(1, E, 2)])

    u = np.arange(E // 2, dtype=np.float64)
    thetas = 10000.0 ** (-2.0 * u / E)
    ang = np.arange(T, dtype=np.float64)[:, None] * thetas[None, :]
    # sin_h[p, half, u, t'] = sin(ang[1024*half + t', u*128 + p])
    sin_h = np.ascontiguousarray(
        np.sin(ang).T.reshape(4, P, 2, 1024).transpose(1, 2, 0, 3)
    ).astype(bf16)
    cos_h = np.ascontiguousarray(
        np.cos(ang).T.reshape(4, P, 2, 1024).transpose(1, 2, 0, 3)
    ).astype(bf16)

    f = np.arange(P)
    mask = (f[None, :] >= f[:, None]).astype(np.float32).astype(bf16)

    xt = []
    for b in range(B):
        xc = input[b].T[perm].reshape(EC, P, NT, P)
        xt.append(np.ascontiguousarray(xc.transpose(2, 1, 0, 3)).astype(bf16))
    WqT, WkT, WvT = Wq.T[perm], Wk.T[perm], Wv.T[perm]

    def wslice(WT, g):
        w = WT[:, DL * g : DL * (g + 1)].reshape(EC, P, DL)
        return np.ascontiguousarray(w.transpose(1, 0, 2)).astype(bf16)

    wq_g = [wslice(WqT, g) for g in range(G)]
    wk_g = [wslice(WkT, g) for g in range(G)]
    wv_g = [wslice(WvT, g) for g in range(G)]
    wo_g = [
        np.ascontiguousarray(
            Wo.T[DL * g : DL * (g + 1)].reshape(DC, P, E).transpose(1, 0, 2)
        ).astype(bf16)
        for g in range(G)
    ]

    in_maps = []
    for c in range(8):
        b, g = c // 2, c % 2
        in_maps.append(
            {
                "xt": xt[b],
                "sin": sin_h,
                "cos": cos_h,
                "wq": wq_g[g],
                "wk": wk_g[g],
                "wv": wv_g[g],
                "wo": wo_g[g],
                "mask": mask,
            }
        )
    return in_maps


def kernel(input, Wq, Wk, Wv, Wo, bo):
    global LAST_RESULT
    input = np.asarray(input, np.float32)
    Wq, Wk, Wv, Wo = (np.asarray(w, np.float32) for w in (Wq, Wk, Wv, Wo))
    bo = np.asarray(bo, np.float32)

    if "nc" not in _CACHE:
        _CACHE["nc"] = _build()
    nc = _CACHE["nc"]

    in_maps = _prep_inputs(input, Wq, Wk, Wv, Wo)
    res = bass_utils.run_bass_kernel_spmd(nc, in_maps, core_ids=list(range(8)))
    LAST_RESULT = res

    out = np.empty((B, T, E), np.float32)
    for b in range(B):
        out[b] = (
            res.results[2 * b]["out"].astype(np.float32)
            + res.results[2 * b + 1]["out"].astype(np.float32)
            + bo
        )
    return out


# revision 45
# speedup vs baseline: 1.2006x; 1.2006x over previous
"""Causal multi-head self-attention (RoPE on input) for Trainium2, 8 NeuronCores.

Sharding: core c handles batch b = c//2 and head-group g = c%2 (8 of 16 heads).
Wq/Wk/Wv are split column-wise per head-group, Wo row-wise; each core produces a
partial (T, E) output and the host sums the two head-group partials per batch
and adds the bias.

v2 design notes (vs the phase-separated v1):
- All host-side tensors are laid out so every DMA is contiguous per
  partition line (no on-the-fly rearranges -> ~10x fewer descriptors).
- RoPE runs in place on the xt tile (products into temps first, then the
  pair-combine writes back), saving a 32KB/partition rx copy.
- The whole kernel is one software pipeline over query blocks i (512
  tokens): per (i, hp) the Q/K projection for token block i is emitted,
  then scores+exp+mask with PV trailing by 2 key tiles; the output
  projection for block i-1 is interleaved into block i so there is no
  serial phase C tail.
- Diagonal score/PV matmuls stream only the causally-live query suffix,
  and the mask multiply covers only the 128-wide triangle column block.
- Softmax normalization: denominators come from a ones column in V; a
  fast approximate reciprocal per pair is broadcast across partitions
  via a DRAM bounce, staggered two pairs behind the attention pipeline.
"""

import numpy as np
import ml_dtypes

import concourse.bacc as bacc
import concourse.tile as tile
import concourse.mybir as mybir
from concourse import bass_utils
from concourse.bass_interp import get_hw_module

bf16 = ml_dtypes.bfloat16
BF = mybir.dt.bfloat16
F32 = mybir.dt.float32
EXP = mybir.ActivationFunctionType.Exp

B, T, E = 4, 2048, 1024
H, HD = 16, 64
G = 2  # head groups (tensor-parallel dimension)
HL = H // G  # heads per core
DL = HL * HD  # 512 local feature dim
P = 128
NT = T // P  # 16 token tiles
NQ = T // 512  # 4 query blocks
EC = E // P  # 8 contraction chunks over E
DC = DL // P  # 4 chunks over local head dims (one per head pair)

_CACHE = {}
LAST_RESULT = None


def _build():
    nc = bacc.Bacc("TRN2", target_bir_lowering=False, debug=False, num_devices=8)
    xt_d = nc.dram_tensor("xt", (NT, P, EC, P), BF, kind="ExternalInput").ap()
    sin_d = nc.dram_tensor("sin", (P, 2, 4, 1024), BF, kind="ExternalInput").ap()
    cos_d = nc.dram_tensor("cos", (P, 2, 4, 1024), BF, kind="ExternalInput").ap()
    wq_d = nc.dram_tensor("wq", (P, EC, DL), BF, kind="ExternalInput").ap()
    wk_d = nc.dram_tensor("wk", (P, EC, DL), BF, kind="ExternalInput").ap()
    wv_d = nc.dram_tensor("wv", (P, EC, DL), BF, kind="ExternalInput").ap()
    wo_d = nc.dram_tensor("wo", (P, DC, E), BF, kind="ExternalInput").ap()
    mask_d = nc.dram_tensor("mask", (P, P), BF, kind="ExternalInput").ap()
    out_d = nc.dram_tensor("out", (T, E), BF, kind="ExternalOutput").ap()

    with tile.TileContext(nc) as tc:
        with (
            tc.tile_pool(name="persist", bufs=1) as persist,
            tc.tile_pool(name="tabs", bufs=1) as tabs,
            tc.tile_pool(name="tmps", bufs=1) as tmps,
            tc.tile_pool(name="att", bufs=6) as apool,
            tc.tile_pool(name="pks", bufs=2) as pkp,
            tc.tile_pool(name="rbs", bufs=3) as rbp,
            tc.tile_pool(name="ots", bufs=4) as otp,
            tc.tile_pool(name="dramn", bufs=3, space="DRAM") as dpool,
            tc.tile_pool(name="ps512", bufs=2, space="PSUM") as ps512,
            tc.tile_pool(name="sps", bufs=2, space="PSUM") as spool,
            tc.tile_pool(name="ops", bufs=2, space="PSUM") as opool,
        ):
            xt = persist.tile([P, NT, EC, P], BF)
            qT = persist.tile([P, DC, T], BF)
            kT = persist.tile([P, DC, T], BF)
            v = persist.tile([P, NT, HL, HD + 1], BF)
            oc = persist.tile([P, DC, T], BF)
            wq = persist.tile([P, EC, DL], BF)
            wk = persist.tile([P, EC, DL], BF)
            wv = persist.tile([P, EC, DL], BF)
            wo = persist.tile([P, DC, E], BF)
            mask = persist.tile([P, P], BF)
            sel = persist.tile([1, HD], BF)

            sins = [tabs.tile([P, 4, 1024], BF, name=f"s{h}") for h in range(2)]
            coss = [tabs.tile([P, 4, 1024], BF, name=f"c{h}") for h in range(2)]

            # ---- DMA emission = approximate arrival order; sequenced so
            # each consumer unblocks as early as possible: V-proj needs
            # wv+xt[k]; RoPE chunk 0 needs xt[0:4] + the per-u table slices;
            # the first K-proj needs wk + RoPE chunk 0.
            # Spread DMA issue over the two HWDGE queues (each issue costs
            # ~650ns serial on its queue; gpsimd/SWDGE generates descriptors
            # in ucode and is far too slow for bulk transfers).
            nc.sync.dma_start(wv[:, 0:2, :], wv_d[:, 0:2])
            nc.sync.dma_start(xt[:, 0:1], xt_d[0:1].rearrange("t p e c -> p t e c"))
            nc.sync.dma_start(wv[:, 2:4, :], wv_d[:, 2:4])
            nc.scalar.dma_start(sins[0][:, 2, :], sin_d[:, 0, 2])
            nc.scalar.dma_start(coss[0][:, 2, :], cos_d[:, 0, 2])
            nc.sync.dma_start(wv[:, 4:8, :], wv_d[:, 4:8])
            nc.scalar.dma_start(sins[0][:, 3, :], sin_d[:, 0, 3])
            nc.scalar.dma_start(coss[0][:, 3, :], cos_d[:, 0, 3])
            nc.sync.dma_start(sins[0][:, 0, :], sin_d[:, 0, 0])
            nc.sync.dma_start(coss[0][:, 0, :], cos_d[:, 0, 0])
            nc.scalar.dma_start(sins[0][:, 1, :], sin_d[:, 0, 1])
            nc.scalar.dma_start(coss[0][:, 1, :], cos_d[:, 0, 1])
            nc.sync.dma_start(xt[:, 1:2], xt_d[1:2].rearrange("t p e c -> p t e c"))
            nc.sync.dma_start(xt[:, 2:4], xt_d[2:4].rearrange("t p e c -> p t e c"))
            nc.scalar.dma_start(wq, wq_d)
            nc.sync.dma_start(xt[:, 4:6], xt_d[4:6].rearrange("t p e c -> p t e c"))
            nc.sync.dma_start(xt[:, 6:8], xt_d[6:8].rearrange("t p e c -> p t e c"))

            nc.vector.memset(v[:, :, :, HD : HD + 1], 1.0)
            nc.vector.memset(sel, 1.0)

            def emit_vproj(tk):
                vp = ps512.tile([P, DL], F32, tag="ps512")
                for j in range(EC):
                    nc.tensor.matmul(
                        vp,
                        lhsT=xt[:, tk, j, :],
                        rhs=wv[:, j, :],
                        start=(j == 0),
                        stop=(j == EC - 1),
                    )
                nc.scalar.copy(
                    v[:, tk, :, 0:HD], vp.rearrange("p (h d) -> p h d", h=HL)
                )

            def emit_rope(tc_):
                """RoPE (in place) for token chunk tc_ (512 tokens). All on
                DVE: offloading u-chunks to GpSimd was tested and reverted --
                concurrent engines on adjacent xt slices contend for SBUF
                ports and slow both to ~3x per-op time."""
                half, qh = tc_ // 2, tc_ % 2
                ts = slice(4 * tc_, 4 * tc_ + 4)
                cs = slice(512 * qh, 512 * qh + 512)
                for u in range(4):
                    # 4 DVE ops per u instead of 6: multiply the (even, odd)
                    # lane pair by cos and sin in one strided op each, then
                    # combine.  xp = xt[:, ts, u::4-stride over (u, u+4)]
                    xe = xt[:, ts, u, :]
                    xo = xt[:, ts, u + 4, :]
                    xp = xt[:, ts, u : u + 5 : 4, :]
                    s_u = sins[half][:, u, cs].rearrange("p (a b) -> p a b", a=4)
                    c_u = coss[half][:, u, cs].rearrange("p (a b) -> p a b", a=4)
                    s_b = s_u[:, :, None, :].to_broadcast((P, 4, 2, P))
                    c_b = c_u[:, :, None, :].to_broadcast((P, 4, 2, P))
                    tc2 = tmps.tile([P, 4, 2, P], BF, tag="tc2")
                    ts2 = tmps.tile([P, 4, 2, P], BF, tag="ts2")
                    nc.vector.tensor_mul(tc2, xp, c_b)
                    nc.vector.tensor_mul(ts2, xp, s_b)
                    nc.vector.tensor_sub(xe, tc2[:, :, 0, :], ts2[:, :, 1, :])
                    nc.vector.tensor_add(xo, tc2[:, :, 1, :], ts2[:, :, 0, :])

            def emit_qk(hp, i):
                """Q+K projection for token block i of pair hp."""
                tq = slice(512 * i, 512 * (i + 1))
                # Q first: scores consume fresh qT at j=0, but the fresh kT
                # tiles only at the diagonal (end of the j loop), so the Q
                # copy hides under the K projection matmuls.
                for w_sb, dst in ((wq, qT), (wk, kT)):
                    pp = ps512.tile([P, 512], F32, tag="ps512")
                    for j in range(EC):
                        nc.tensor.matmul(
                            pp,
                            lhsT=w_sb[:, j, P * hp : P * (hp + 1)],
                            rhs=xt[:, 4 * i : 4 * i + 4, j, :],
                            start=(j == 0),
                            stop=(j == EC - 1),
                        )
                    nc.vector.tensor_copy(dst[:, hp, tq], pp)

            def emit_outproj_groups(i, tsub, alt=False):
                """Output projection for token sub-tile tsub (0..3) of block i."""
                tt = 4 * i + tsub
                for et in range(2):
                    pp = ps512.tile([P, 512], F32, tag="ps512")
                    for kk in range(DC):
                        nc.tensor.matmul(
                            pp,
                            lhsT=oc[:, kk, P * tt : P * (tt + 1)],
                            rhs=wo[:, kk, 512 * et : 512 * (et + 1)],
                            start=(kk == 0),
                            stop=(kk == DC - 1),
                        )
                    ot = otp.tile([P, 512], BF, tag="ot")
                    if alt and et == 0:
                        nc.scalar.copy(ot, pp)
                    else:
                        nc.vector.tensor_copy(ot, pp)
                    eng = nc.scalar if (alt and et == 1) else nc.sync
                    eng.dma_start(
                        out_d[P * tt : P * (tt + 1), 512 * et : 512 * (et + 1)], ot
                    )

            def emit_attn(hp, i):
                """Scores+exp+mask with trailing PV for (block i, pair hp)."""
                h0, h1 = 2 * hp, 2 * hp + 1
                nj = 4 * i + 4
                tq0 = 512 * i
                op0 = opool.tile([HD + 1, 512], F32, tag="o")
                op1 = opool.tile([HD + 1, 512], F32, tag="o")
                ats = []

                def emit_pv(jp, at_jp):
                    lo = max(0, P * (jp - 4 * i))
                    nc.tensor.matmul(
                        op0[:, lo:512],
                        lhsT=v[:, jp, h0, :],
                        rhs=at_jp[:, 0, lo:512],
                        start=(jp == 0),
                        stop=(jp == nj - 1),
                    )
                    nc.tensor.matmul(
                        op1[:, lo:512],
                        lhsT=v[:, jp, h1, :],
                        rhs=at_jp[:, 1, lo:512],
                        start=(jp == 0),
                        stop=(jp == nj - 1),
                    )

                for j in range(nj):
                    r = j - 4 * i
                    lo = max(0, P * r)
                    sp = spool.tile([P, 2, 512], F32, tag="s")
                    nc.tensor.matmul(
                        sp[:, 0, lo:512],
                        lhsT=kT[0:HD, hp, P * j : P * (j + 1)],
                        rhs=qT[0:HD, hp, tq0 + lo : tq0 + 512],
                        start=True,
                        stop=True,
                    )
                    nc.tensor.matmul(
                        sp[:, 1, lo:512],
                        lhsT=kT[HD:P, hp, P * j : P * (j + 1)],
                        rhs=qT[HD:P, hp, tq0 + lo : tq0 + 512],
                        start=True,
                        stop=True,
                    )
                    at = apool.tile([P, 2, 512], BF, tag="a")
                    nc.scalar.activation(
                        at[:, :, lo:512], sp[:, :, lo:512], EXP, scale=0.125
                    )
                    if r >= 0:
                        # only the 128-wide diagonal block needs the triangle
                        # mask; columns beyond it are fully live
                        nc.vector.tensor_mul(
                            at[:, :, lo : lo + P],
                            at[:, :, lo : lo + P],
                            mask[:, None, :].to_broadcast((P, 2, P)),
                        )
                    ats.append(at)
                    if j >= 2:
                        emit_pv(j - 2, ats[j - 2])
                for jp in range(max(nj - 2, 0), nj):
                    emit_pv(jp, ats[jp])

                # denominators (ones-column rows) -> staging at partition 0
                # (engine APs need aligned partition bases)
                s0 = pkp.tile([1, 512], F32, tag="sd", bufs=4)
                s1 = pkp.tile([1, 512], F32, tag="sd", bufs=4)
                nc.scalar.copy(s0, op0[HD : HD + 1, :])
                nc.scalar.copy(s1, op1[HD : HD + 1, :])
                # oc copies are deferred into finish_norm / finish_norm_fast
                # (one pair later) so the next pair's fresh Q copy goes first
                # on the DVE queue
                return s0, s1, op0, op1

            def finish_norm(i, hp, s0, s1, op0, op1):
                """Normalize oc for (i, hp): reciprocal rows broadcast across
                partitions via two K=1 matmuls (ones(1,64) x row) into one
                PSUM tile, then a single fused multiply into oc. Emitted one
                pair later so the PE never waits on the reciprocal chain."""
                tq = slice(512 * i, 512 * (i + 1))
                nc.vector.tensor_copy(oc[0:HD, hp, tq], op0[0:HD, :])
                nc.vector.tensor_copy(oc[HD:P, hp, tq], op1[0:HD, :])
                r0 = pkp.tile([1, 512], F32, tag="re", bufs=2)
                r1 = pkp.tile([1, 512], F32, tag="re", bufs=2)
                nc.vector.reciprocal_approx_fast(r0, s0)
                nc.vector.reciprocal_approx_fast(r1, s1)
                rd = dpool.tile([2, 512], F32, tag="rd", name=f"rd{i}_{hp}")
                nc.sync.dma_start(rd[0:1, :], r0)
                nc.sync.dma_start(rd[1:2, :], r1)
                rb = rbp.tile([P, 512], F32, tag="rb")
                nc.sync.dma_start(rb[0:HD, :], rd[0:1, :].to_broadcast((HD, 512)))
                nc.sync.dma_start(rb[HD:P, :], rd[1:2, :].to_broadcast((HD, 512)))
                return rb

            def finish_mul(i, hp, rb):
                tq = slice(512 * i, 512 * (i + 1))
                nc.vector.tensor_mul(oc[:, hp, tq], oc[:, hp, tq], rb)

            def finish_norm_fast(i, hp, s0, s1, op0, op1):
                """Tail-only normalize: broadcast via two K=1 PE matmuls
                (ones(1,64) x reciprocal row) instead of the DMA bounce --
                the PE is idle at the tail, and this chain is ~4us shorter."""
                tq = slice(512 * i, 512 * (i + 1))
                r0 = pkp.tile([1, 512], F32, tag="re", bufs=2)
                r1 = pkp.tile([1, 512], F32, tag="re", bufs=2)
                nc.vector.reciprocal_approx_fast(r0, s0)
                nc.vector.reciprocal_approx_fast(r1, s1)
                rc0 = pkp.tile([1, 512], BF, tag="rc", bufs=2)
                rc1 = pkp.tile([1, 512], BF, tag="rc", bufs=2)
                nc.vector.tensor_copy(rc0, r0)
                nc.vector.tensor_copy(rc1, r1)
                nc.scalar.copy(oc[0:HD, hp, tq], op0[0:HD, :])
                nc.scalar.copy(oc[HD:P, hp, tq], op1[0:HD, :])
                rb = ps512.tile([P, 512], F32, tag="ps512")
                nc.tensor.matmul(rb[0:HD, :], lhsT=sel, rhs=rc0, start=True, stop=True)
                nc.tensor.matmul(rb[HD:P, :], lhsT=sel, rhs=rc1, start=True, stop=True)
                nc.vector.tensor_mul(oc[:, hp, tq], oc[:, hp, tq], rb)

            # ---- pipeline ----
            for tk in range(6):
                emit_vproj(tk)
            nc.scalar.dma_start(mask, mask_d)
            emit_vproj(6)
            emit_vproj(7)
            # late-needed bytes (~6MB): issued from the scalar queue behind
            # the V-proj copies so their descriptors don't steal bandwidth
            # from the startup-critical set
            nc.scalar.dma_start(wk, wk_d)
            nc.scalar.dma_start(xt[:, 8:12], xt_d[8:12].rearrange("t p e c -> p t e c"))
            nc.scalar.dma_start(xt[:, 12:16], xt_d[12:16].rearrange("t p e c -> p t e c"))
            nc.scalar.dma_start(sins[1], sin_d[:, 1])
            nc.scalar.dma_start(coss[1], cos_d[:, 1])
            nc.scalar.dma_start(wo, wo_d)
            emit_rope(0)

            bounce_q = None  # (i, hp, s0, s1): reciprocal+broadcast next pair
            mul_q = None  # (i, hp, rb): normalize-multiply the pair after
            for i in range(NQ):
                for hp in range(DC):
                    emit_qk(hp, i)
                    if mul_q is not None:
                        finish_mul(*mul_q)
                        mul_q = None
                    if bounce_q is not None:
                        bi, bhp = bounce_q[0], bounce_q[1]
                        mul_q = (bi, bhp, finish_norm(*bounce_q))
                    bounce_q = (i, hp) + emit_attn(hp, i)
                    if i == 0:
                        emit_vproj(8 + 2 * hp)
                        emit_vproj(9 + 2 * hp)
                    if hp == 2 and i < 3:
                        emit_rope(i + 1)
                    if i > 0 and hp >= 1:
                        emit_outproj_groups(i - 1, hp - 1)
                        if hp == 3:
                            emit_outproj_groups(i - 1, 3)
            bi, bhp = bounce_q[0], bounce_q[1]
            finish_mul(*mul_q)
            finish_norm_fast(bi, bhp, *bounce_q[2:])
            for tsub in range(3):
                emit_outproj_groups(NQ - 1, tsub, alt=True)
            # last token tile: 256-wide units so the final copy->DMA chain
            # drains at half granularity
            tt = 4 * (NQ - 1) + 3
            for eq in range(4):
                pp = ps512.tile([P, 256], F32, tag="ps512")
                for kk in range(DC):
                    nc.tensor.matmul(
                        pp,
                        lhsT=oc[:, kk, P * tt : P * (tt + 1)],
                        rhs=wo[:, kk, 256 * eq : 256 * (eq + 1)],
                        start=(kk == 0),
                        stop=(kk == DC - 1),
                    )
                ot = otp.tile([P, 256], BF, tag="ot")
                if eq % 2 == 0:
                    nc.scalar.copy(ot, pp)
                else:
                    nc.vector.tensor_copy(ot, pp)
                eng = nc.scalar if eq % 2 else nc.sync
                eng.dma_start(
                    out_d[P * tt : P * (tt + 1), 256 * eq : 256 * (eq + 1)], ot
                )

    nc.compile()
    nc.m = get_hw_module(nc.m)
    return nc


def _prep_inputs(input, Wq, Wk, Wv, Wo):
    """Host-side shard prep: transpose/de-interleave/cast. Returns 8 in_maps."""
    perm = np.concatenate([np.arange(0, E, 2), np.arange(1, E, 2)])

    u = np.arange(E // 2, dtype=np.float64)
    thetas = 10000.0 ** (-2.0 * u / E)
    ang = np.arange(T, dtype=np.float64)[:, None] * thetas[None, :]
    # sin_h[p, half, u, t'] = sin(ang[1024*half + t', u*128 + p])
    sin_h = np.ascontiguousarray(
        np.sin(ang).T.reshape(4, P, 2, 1024).transpose(1, 2, 0, 3)
    ).astype(bf16)
    cos_h = np.ascontiguousarray(
        np.cos(ang).T.reshape(4, P, 2, 1024).transpose(1, 2, 0, 3)
    ).astype(bf16)

    f = np.arange(P)
    mask = (f[None, :] >= f[:, None]).astype(np.float32).astype(bf16)

    xt = []
    for b in range(B):
        xc = input[b].T[perm].reshape(EC, P, NT, P)
        xt.append(np.ascontiguousarray(xc.transpose(2, 1, 0, 3)).astype(bf16))
    WqT, WkT, WvT = Wq.T[perm], Wk.T[perm], Wv.T[perm]

    def wslice(WT, g):
        w = WT[:, DL * g : DL * (g + 1)].reshape(EC, P, DL)
        return np.ascontiguousarray(w.transpose(1, 0, 2)).astype(bf16)

    wq_g = [wslice(WqT, g) for g in range(G)]
    wk_g = [wslice(WkT, g) for g in range(G)]
    wv_g = [wslice(WvT, g) for g in range(G)]
    wo_g = [
        np.ascontiguousarray(
            Wo.T[DL * g : DL * (g + 1)].reshape(DC, P, E).transpose(1, 0, 2)
        ).astype(bf16)
        for g in range(G)
    ]

    in_maps = []
    for c in range(8):
        b, g = c // 2, c % 2
        in_maps.append(
            {
                "xt": xt[b],
                "sin": sin_h,
                "cos": cos_h,
                "wq": wq_g[g],
                "wk": wk_g[g],
                "wv": wv_g[g],
                "wo": wo_g[g],
                "mask": mask,
            }
        )
    return in_maps


def kernel(input, Wq, Wk, Wv, Wo, bo):
    global LAST_RESULT
    input = np.asarray(input, np.float32)
    Wq, Wk, Wv, Wo = (np.asarray(w, np.float32) for w in (Wq, Wk, Wv, Wo))
    bo = np.asarray(bo, np.float32)

    if "nc" not in _CACHE:
        _CACHE["nc"] = _build()
    nc = _CACHE["nc"]

    in_maps = _prep_inputs(input, Wq, Wk, Wv, Wo)
    res = bass_utils.run_bass_kernel_spmd(nc, in_maps, core_ids=list(range(8)))
    LAST_RESULT = res

    out = np.empty((B, T, E), np.float32)
    for b in range(B):
        out[b] = (
            res.results[2 * b]["out"].astype(np.float32)
            + res.results[2 * b + 1]["out"].astype(np.float32)
            + bo
        )
    return out


# revision 46
# speedup vs baseline: 1.2082x; 1.0063x over previous
"""Causal multi-head self-attention (RoPE on input) for Trainium2, 8 NeuronCores.

Sharding: core c handles batch b = c//2 and head-group g = c%2 (8 of 16 heads).
Wq/Wk/Wv are split column-wise per head-group, Wo row-wise; each core produces a
partial (T, E) output and the host sums the two head-group partials per batch
and adds the bias.

v2 design notes (vs the phase-separated v1):
- All host-side tensors are laid out so every DMA is contiguous per
  partition line (no on-the-fly rearranges -> ~10x fewer descriptors).
- RoPE runs in place on the xt tile (products into temps first, then the
  pair-combine writes back), saving a 32KB/partition rx copy.
- The whole kernel is one software pipeline over query blocks i (512
  tokens): per (i, hp) the Q/K projection for token block i is emitted,
  then scores+exp+mask with PV trailing by 2 key tiles; the output
  projection for block i-1 is interleaved into block i so there is no
  serial phase C tail.
- Diagonal score/PV matmuls stream only the causally-live query suffix,
  and the mask multiply covers only the 128-wide triangle column block.
- Softmax normalization: denominators come from a ones column in V; a
  fast approximate reciprocal per pair is broadcast across partitions
  via a DRAM bounce, staggered two pairs behind the attention pipeline.
"""

import numpy as np
import ml_dtypes

import concourse.bacc as bacc
import concourse.tile as tile
import concourse.mybir as mybir
from concourse import bass_utils
from concourse.bass_interp import get_hw_module

bf16 = ml_dtypes.bfloat16
BF = mybir.dt.bfloat16
F32 = mybir.dt.float32
EXP = mybir.ActivationFunctionType.Exp

B, T, E = 4, 2048, 1024
H, HD = 16, 64
G = 2  # head groups (tensor-parallel dimension)
HL = H // G  # heads per core
DL = HL * HD  # 512 local feature dim
P = 128
NT = T // P  # 16 token tiles
NQ = T // 512  # 4 query blocks
EC = E // P  # 8 contraction chunks over E
DC = DL // P  # 4 chunks over local head dims (one per head pair)

_CACHE = {}
LAST_RESULT = None


def _build():
    nc = bacc.Bacc("TRN2", target_bir_lowering=False, debug=False, num_devices=8)
    xt_d = nc.dram_tensor("xt", (NT, P, EC, P), BF, kind="ExternalInput").ap()
    sin_d = nc.dram_tensor("sin", (P, 2, 4, 1024), BF, kind="ExternalInput").ap()
    cos_d = nc.dram_tensor("cos", (P, 2, 4, 1024), BF, kind="ExternalInput").ap()
    wq_d = nc.dram_tensor("wq", (P, EC, DL), BF, kind="ExternalInput").ap()
    wk_d = nc.dram_tensor("wk", (P, EC, DL), BF, kind="ExternalInput").ap()
    wv_d = nc.dram_tensor("wv", (P, EC, DL), BF, kind="ExternalInput").ap()
    wo_d = nc.dram_tensor("wo", (P, DC, E), BF, kind="ExternalInput").ap()
    mask_d = nc.dram_tensor("mask", (P, P), BF, kind="ExternalInput").ap()
    out_d = nc.dram_tensor("out", (T, E), BF, kind="ExternalOutput").ap()

    with tile.TileContext(nc) as tc:
        with (
            tc.tile_pool(name="persist", bufs=1) as persist,
            tc.tile_pool(name="tabs", bufs=1) as tabs,
            tc.tile_pool(name="tmps", bufs=1) as tmps,
            tc.tile_pool(name="att", bufs=6) as apool,
            tc.tile_pool(name="pks", bufs=2) as pkp,
            tc.tile_pool(name="rbs", bufs=3) as rbp,
            tc.tile_pool(name="ots", bufs=4) as otp,
            tc.tile_pool(name="dramn", bufs=3, space="DRAM") as dpool,
            tc.tile_pool(name="ps512", bufs=2, space="PSUM") as ps512,
            tc.tile_pool(name="sps", bufs=2, space="PSUM") as spool,
            tc.tile_pool(name="ops", bufs=2, space="PSUM") as opool,
        ):
            xt = persist.tile([P, NT, EC, P], BF)
            qT = persist.tile([P, DC, T], BF)
            kT = persist.tile([P, DC, T], BF)
            v = persist.tile([P, NT, HL, HD + 1], BF)
            oc = persist.tile([P, DC, T], BF)
            wq = persist.tile([P, EC, DL], BF)
            wk = persist.tile([P, EC, DL], BF)
            wv = persist.tile([P, EC, DL], BF)
            wo = persist.tile([P, DC, E], BF)
            mask = persist.tile([P, P], BF)
            sel = persist.tile([1, HD], BF)

            sins = [tabs.tile([P, 4, 1024], BF, name=f"s{h}") for h in range(2)]
            coss = [tabs.tile([P, 4, 1024], BF, name=f"c{h}") for h in range(2)]

            # ---- DMA emission = approximate arrival order; sequenced so
            # each consumer unblocks as early as possible: V-proj needs
            # wv+xt[k]; RoPE chunk 0 needs xt[0:4] + the per-u table slices;
            # the first K-proj needs wk + RoPE chunk 0.
            # Spread DMA issue over the two HWDGE queues (each issue costs
            # ~650ns serial on its queue; gpsimd/SWDGE generates descriptors
            # in ucode and is far too slow for bulk transfers).
            nc.sync.dma_start(wv[:, 0:2, :], wv_d[:, 0:2])
            nc.sync.dma_start(xt[:, 0:1], xt_d[0:1].rearrange("t p e c -> p t e c"))
            nc.sync.dma_start(wv[:, 2:4, :], wv_d[:, 2:4])
            nc.scalar.dma_start(sins[0][:, 2, :], sin_d[:, 0, 2])
            nc.scalar.dma_start(coss[0][:, 2, :], cos_d[:, 0, 2])
            nc.sync.dma_start(wv[:, 4:8, :], wv_d[:, 4:8])
            nc.scalar.dma_start(sins[0][:, 3, :], sin_d[:, 0, 3])
            nc.scalar.dma_start(coss[0][:, 3, :], cos_d[:, 0, 3])
            nc.sync.dma_start(xt[:, 1:2], xt_d[1:2].rearrange("t p e c -> p t e c"))
            nc.sync.dma_start(xt[:, 2:4], xt_d[2:4].rearrange("t p e c -> p t e c"))
            nc.sync.dma_start(sins[0][:, 0, :], sin_d[:, 0, 0])
            nc.sync.dma_start(coss[0][:, 0, :], cos_d[:, 0, 0])
            nc.scalar.dma_start(sins[0][:, 1, :], sin_d[:, 0, 1])
            nc.scalar.dma_start(coss[0][:, 1, :], cos_d[:, 0, 1])
            nc.scalar.dma_start(wq, wq_d)
            nc.sync.dma_start(xt[:, 4:6], xt_d[4:6].rearrange("t p e c -> p t e c"))
            nc.sync.dma_start(xt[:, 6:8], xt_d[6:8].rearrange("t p e c -> p t e c"))

            nc.vector.memset(v[:, :, :, HD : HD + 1], 1.0)
            nc.vector.memset(sel, 1.0)

            def emit_vproj(tk):
                vp = ps512.tile([P, DL], F32, tag="ps512")
                for j in range(EC):
                    nc.tensor.matmul(
                        vp,
                        lhsT=xt[:, tk, j, :],
                        rhs=wv[:, j, :],
                        start=(j == 0),
                        stop=(j == EC - 1),
                    )
                nc.scalar.copy(
                    v[:, tk, :, 0:HD], vp.rearrange("p (h d) -> p h d", h=HL)
                )

            def emit_rope(tc_):
                """RoPE (in place) for token chunk tc_ (512 tokens). All on
                DVE: offloading u-chunks to GpSimd was tested and reverted --
                concurrent engines on adjacent xt slices contend for SBUF
                ports and slow both to ~3x per-op time."""
                half, qh = tc_ // 2, tc_ % 2
                ts = slice(4 * tc_, 4 * tc_ + 4)
                cs = slice(512 * qh, 512 * qh + 512)
                for u in range(4):
                    # 4 DVE ops per u instead of 6: multiply the (even, odd)
                    # lane pair by cos and sin in one strided op each, then
                    # combine.  xp = xt[:, ts, u::4-stride over (u, u+4)]
                    xe = xt[:, ts, u, :]
                    xo = xt[:, ts, u + 4, :]
                    xp = xt[:, ts, u : u + 5 : 4, :]
                    s_u = sins[half][:, u, cs].rearrange("p (a b) -> p a b", a=4)
                    c_u = coss[half][:, u, cs].rearrange("p (a b) -> p a b", a=4)
                    s_b = s_u[:, :, None, :].to_broadcast((P, 4, 2, P))
                    c_b = c_u[:, :, None, :].to_broadcast((P, 4, 2, P))
                    tc2 = tmps.tile([P, 4, 2, P], BF, tag="tc2")
                    ts2 = tmps.tile([P, 4, 2, P], BF, tag="ts2")
                    nc.vector.tensor_mul(tc2, xp, c_b)
                    nc.vector.tensor_mul(ts2, xp, s_b)
                    nc.vector.tensor_sub(xe, tc2[:, :, 0, :], ts2[:, :, 1, :])
                    nc.vector.tensor_add(xo, tc2[:, :, 1, :], ts2[:, :, 0, :])

            def emit_qk(hp, i):
                """Q+K projection for token block i of pair hp."""
                tq = slice(512 * i, 512 * (i + 1))
                # Q first: scores consume fresh qT at j=0, but the fresh kT
                # tiles only at the diagonal (end of the j loop), so the Q
                # copy hides under the K projection matmuls.
                for w_sb, dst in ((wq, qT), (wk, kT)):
                    pp = ps512.tile([P, 512], F32, tag="ps512")
                    for j in range(EC):
                        nc.tensor.matmul(
                            pp,
                            lhsT=w_sb[:, j, P * hp : P * (hp + 1)],
                            rhs=xt[:, 4 * i : 4 * i + 4, j, :],
                            start=(j == 0),
                            stop=(j == EC - 1),
                        )
                    nc.vector.tensor_copy(dst[:, hp, tq], pp)

            def emit_outproj_groups(i, tsub, alt=False):
                """Output projection for token sub-tile tsub (0..3) of block i."""
                tt = 4 * i + tsub
                for et in range(2):
                    pp = ps512.tile([P, 512], F32, tag="ps512")
                    for kk in range(DC):
                        nc.tensor.matmul(
                            pp,
                            lhsT=oc[:, kk, P * tt : P * (tt + 1)],
                            rhs=wo[:, kk, 512 * et : 512 * (et + 1)],
                            start=(kk == 0),
                            stop=(kk == DC - 1),
                        )
                    ot = otp.tile([P, 512], BF, tag="ot")
                    if alt and et == 0:
                        nc.scalar.copy(ot, pp)
                    else:
                        nc.vector.tensor_copy(ot, pp)
                    eng = nc.scalar if (alt and et == 1) else nc.sync
                    eng.dma_start(
                        out_d[P * tt : P * (tt + 1), 512 * et : 512 * (et + 1)], ot
                    )

            def emit_attn(hp, i):
                """Scores+exp+mask with trailing PV for (block i, pair hp)."""
                h0, h1 = 2 * hp, 2 * hp + 1
                nj = 4 * i + 4
                tq0 = 512 * i
                op0 = opool.tile([HD + 1, 512], F32, tag="o")
                op1 = opool.tile([HD + 1, 512], F32, tag="o")
                ats = []

                def emit_pv(jp, at_jp):
                    lo = max(0, P * (jp - 4 * i))
                    nc.tensor.matmul(
                        op0[:, lo:512],
                        lhsT=v[:, jp, h0, :],
                        rhs=at_jp[:, 0, lo:512],
                        start=(jp == 0),
                        stop=(jp == nj - 1),
                    )
                    nc.tensor.matmul(
                        op1[:, lo:512],
                        lhsT=v[:, jp, h1, :],
                        rhs=at_jp[:, 1, lo:512],
                        start=(jp == 0),
                        stop=(jp == nj - 1),
                    )

                for j in range(nj):
                    r = j - 4 * i
                    lo = max(0, P * r)
                    sp = spool.tile([P, 2, 512], F32, tag="s")
                    nc.tensor.matmul(
                        sp[:, 0, lo:512],
                        lhsT=kT[0:HD, hp, P * j : P * (j + 1)],
                        rhs=qT[0:HD, hp, tq0 + lo : tq0 + 512],
                        start=True,
                        stop=True,
                    )
                    nc.tensor.matmul(
                        sp[:, 1, lo:512],
                        lhsT=kT[HD:P, hp, P * j : P * (j + 1)],
                        rhs=qT[HD:P, hp, tq0 + lo : tq0 + 512],
                        start=True,
                        stop=True,
                    )
                    at = apool.tile([P, 2, 512], BF, tag="a")
                    nc.scalar.activation(
                        at[:, :, lo:512], sp[:, :, lo:512], EXP, scale=0.125
                    )
                    if r >= 0:
                        # only the 128-wide diagonal block needs the triangle
                        # mask; columns beyond it are fully live
                        nc.vector.tensor_mul(
                            at[:, :, lo : lo + P],
                            at[:, :, lo : lo + P],
                            mask[:, None, :].to_broadcast((P, 2, P)),
                        )
                    ats.append(at)
                    if j >= 2:
                        emit_pv(j - 2, ats[j - 2])
                for jp in range(max(nj - 2, 0), nj):
                    emit_pv(jp, ats[jp])

                # denominators (ones-column rows) -> staging at partition 0
                # (engine APs need aligned partition bases)
                s0 = pkp.tile([1, 512], F32, tag="sd", bufs=4)
                s1 = pkp.tile([1, 512], F32, tag="sd", bufs=4)
                nc.scalar.copy(s0, op0[HD : HD + 1, :])
                nc.scalar.copy(s1, op1[HD : HD + 1, :])
                # oc copies are deferred into finish_norm / finish_norm_fast
                # (one pair later) so the next pair's fresh Q copy goes first
                # on the DVE queue
                return s0, s1, op0, op1

            def finish_norm(i, hp, s0, s1, op0, op1):
                """Normalize oc for (i, hp): reciprocal rows broadcast across
                partitions via two K=1 matmuls (ones(1,64) x row) into one
                PSUM tile, then a single fused multiply into oc. Emitted one
                pair later so the PE never waits on the reciprocal chain."""
                tq = slice(512 * i, 512 * (i + 1))
                nc.vector.tensor_copy(oc[0:HD, hp, tq], op0[0:HD, :])
                nc.vector.tensor_copy(oc[HD:P, hp, tq], op1[0:HD, :])
                r0 = pkp.tile([1, 512], F32, tag="re", bufs=2)
                r1 = pkp.tile([1, 512], F32, tag="re", bufs=2)
                nc.vector.reciprocal_approx_fast(r0, s0)
                nc.vector.reciprocal_approx_fast(r1, s1)
                rd = dpool.tile([2, 512], F32, tag="rd", name=f"rd{i}_{hp}")
                nc.sync.dma_start(rd[0:1, :], r0)
                nc.sync.dma_start(rd[1:2, :], r1)
                rb = rbp.tile([P, 512], F32, tag="rb")
                nc.sync.dma_start(rb[0:HD, :], rd[0:1, :].to_broadcast((HD, 512)))
                nc.sync.dma_start(rb[HD:P, :], rd[1:2, :].to_broadcast((HD, 512)))
                return rb

            def finish_mul(i, hp, rb):
                tq = slice(512 * i, 512 * (i + 1))
                nc.vector.tensor_mul(oc[:, hp, tq], oc[:, hp, tq], rb)

            def finish_norm_fast(i, hp, s0, s1, op0, op1):
                """Tail-only normalize: broadcast via two K=1 PE matmuls
                (ones(1,64) x reciprocal row) instead of the DMA bounce --
                the PE is idle at the tail, and this chain is ~4us shorter."""
                tq = slice(512 * i, 512 * (i + 1))
                r0 = pkp.tile([1, 512], F32, tag="re", bufs=2)
                r1 = pkp.tile([1, 512], F32, tag="re", bufs=2)
                nc.vector.reciprocal_approx_fast(r0, s0)
                nc.vector.reciprocal_approx_fast(r1, s1)
                rc0 = pkp.tile([1, 512], BF, tag="rc", bufs=2)
                rc1 = pkp.tile([1, 512], BF, tag="rc", bufs=2)
                nc.vector.tensor_copy(rc0, r0)
                nc.vector.tensor_copy(rc1, r1)
                nc.scalar.copy(oc[0:HD, hp, tq], op0[0:HD, :])
                nc.scalar.copy(oc[HD:P, hp, tq], op1[0:HD, :])
                rb = ps512.tile([P, 512], F32, tag="ps512")
                nc.tensor.matmul(rb[0:HD, :], lhsT=sel, rhs=rc0, start=True, stop=True)
                nc.tensor.matmul(rb[HD:P, :], lhsT=sel, rhs=rc1, start=True, stop=True)
                nc.vector.tensor_mul(oc[:, hp, tq], oc[:, hp, tq], rb)

            # ---- pipeline ----
            for tk in range(6):
                emit_vproj(tk)
            nc.scalar.dma_start(mask, mask_d)
            emit_vproj(6)
            emit_vproj(7)
            # late-needed bytes (~6MB): issued from the scalar queue behind
            # the V-proj copies so their descriptors don't steal bandwidth
            # from the startup-critical set
            nc.scalar.dma_start(wk, wk_d)
            nc.scalar.dma_start(xt[:, 8:12], xt_d[8:12].rearrange("t p e c -> p t e c"))
            nc.scalar.dma_start(xt[:, 12:16], xt_d[12:16].rearrange("t p e c -> p t e c"))
            nc.scalar.dma_start(sins[1], sin_d[:, 1])
            nc.scalar.dma_start(coss[1], cos_d[:, 1])
            nc.scalar.dma_start(wo, wo_d)
            emit_rope(0)

            bounce_q = None  # (i, hp, s0, s1): reciprocal+broadcast next pair
            mul_q = None  # (i, hp, rb): normalize-multiply the pair after
            for i in range(NQ):
                for hp in range(DC):
                    emit_qk(hp, i)
                    if mul_q is not None:
                        finish_mul(*mul_q)
                        mul_q = None
                    if bounce_q is not None:
                        bi, bhp = bounce_q[0], bounce_q[1]
                        mul_q = (bi, bhp, finish_norm(*bounce_q))
                    bounce_q = (i, hp) + emit_attn(hp, i)
                    if i == 0:
                        emit_vproj(8 + 2 * hp)
                        emit_vproj(9 + 2 * hp)
                    if hp == 2 and i < 3:
                        emit_rope(i + 1)
                    if i > 0 and hp >= 1:
                        emit_outproj_groups(i - 1, hp - 1)
                        if hp == 3:
                            emit_outproj_groups(i - 1, 3)
            bi, bhp = bounce_q[0], bounce_q[1]
            finish_mul(*mul_q)
            finish_norm_fast(bi, bhp, *bounce_q[2:])
            for tsub in range(3):
                emit_outproj_groups(NQ - 1, tsub, alt=True)
            # last token tile: 256-wide units so the final copy->DMA chain
            # drains at half granularity
            tt = 4 * (NQ - 1) + 3
            for eq in range(4):
                pp = ps512.tile([P, 256], F32, tag="ps512")
                for kk in range(DC):
                    nc.tensor.matmul(
                        pp,
                        lhsT=oc[:, kk, P * tt : P * (tt + 1)],
                        rhs=wo[:, kk, 256 * eq : 256 * (eq + 1)],
                        start=(kk == 0),
                        stop=(kk == DC - 1),
                    )
                ot = otp.tile([P, 256], BF, tag="ot")
                if eq % 2 == 0:
                    nc.scalar.copy(ot, pp)
                else:
                    nc.vector.tensor_copy(ot, pp)
                eng = nc.scalar if eq % 2 else nc.sync
                eng.dma_start(
                    out_d[P * tt : P * (tt + 1), 256 * eq : 256 * (eq + 1)], ot
                )

    nc.compile()
    nc.m = get_hw_module(nc.m)
    return nc


def _prep_inputs(input, Wq, Wk, Wv, Wo):
    """Host-side shard prep: transpose/de-interleave/cast. Returns 8 in_maps."""
    perm = np.concatenate([np.arange(0, E, 2), np.arange(1, E, 2)])

    u = np.arange(E // 2, dtype=np.float64)
    thetas = 10000.0 ** (-2.0 * u / E)
    ang = np.arange(T, dtype=np.float64)[:, None] * thetas[None, :]
    # sin_h[p, half, u, t'] = sin(ang[1024*half + t', u*128 + p])
    sin_h = np.ascontiguousarray(
        np.sin(ang).T.reshape(4, P, 2, 1024).transpose(1, 2, 0, 3)
    ).astype(bf16)
    cos_h = np.ascontiguousarray(
        np.cos(ang).T.reshape(4, P, 2, 1024).transpose(1, 2, 0, 3)
    ).astype(bf16)

    f = np.arange(P)
    mask = (f[None, :] >= f[:, None]).astype(np.float32).astype(bf16)

    xt = []
    for b in range(B):
        xc = input[b].T[perm].reshape(EC, P, NT, P)
        xt.append(np.ascontiguousarray(xc.transpose(2, 1, 0, 3)).astype(bf16))
    WqT, WkT, WvT = Wq.T[perm], Wk.T[perm], Wv.T[perm]

    def wslice(WT, g):
        w = WT[:, DL * g : DL * (g + 1)].reshape(EC, P, DL)
        return np.ascontiguousarray(w.transpose(1, 0, 2)).astype(bf16)

    wq_g = [wslice(WqT, g) for g in range(G)]
    wk_g = [wslice(WkT, g) for g in range(G)]
    wv_g = [wslice(WvT, g) for g in range(G)]
    wo_g = [
        np.ascontiguousarray(
            Wo.T[DL * g : DL * (g + 1)].reshape(DC, P, E).transpose(1, 0, 2)
        ).astype(bf16)
        for g in range(G)
    ]

    in_maps = []
    for c in range(8):
        b, g = c // 2, c % 2
        in_maps.append(
            {
                "xt": xt[b],
                "sin": sin_h,
                "cos": cos_h,
                "wq": wq_g[g],
                "wk": wk_g[g],
                "wv": wv_g[g],
                "wo": wo_g[g],
                "mask": mask,
            }
        )
    return in_maps


def kernel(input, Wq, Wk, Wv, Wo, bo):
    global LAST_RESULT
    input = np.asarray(input, np.float32)
    Wq, Wk, Wv, Wo = (np.asarray(w, np.float32) for w in (Wq, Wk, Wv, Wo))
    bo = np.asarray(bo, np.float32)

    if "nc" not in _CACHE:
        _CACHE["nc"] = _build()
    nc = _CACHE["nc"]

    in_maps = _prep_inputs(input, Wq, Wk, Wv, Wo)
    res = bass_utils.run_bass_kernel_spmd(nc, in_maps, core_ids=list(range(8)))
    LAST_RESULT = res

    out = np.empty((B, T, E), np.float32)
    for b in range(B):
        out[b] = (
            res.results[2 * b]["out"].astype(np.float32)
            + res.results[2 * b + 1]["out"].astype(np.float32)
            + bo
        )
    return out


# revision 47
# speedup vs baseline: 1.2083x; 1.0001x over previous
"""Causal multi-head self-attention (RoPE on input) for Trainium2, 8 NeuronCores.

Sharding: core c handles batch b = c//2 and head-group g = c%2 (8 of 16 heads).
Wq/Wk/Wv are split column-wise per head-group, Wo row-wise; each core produces a
partial (T, E) output and the host sums the two head-group partials per batch
and adds the bias.

v2 design notes (vs the phase-separated v1):
- All host-side tensors are laid out so every DMA is contiguous per
  partition line (no on-the-fly rearranges -> ~10x fewer descriptors).
- RoPE runs in place on the xt tile (products into temps first, then the
  pair-combine writes back), saving a 32KB/partition rx copy.
- The whole kernel is one software pipeline over query blocks i (512
  tokens): per (i, hp) the Q/K projection for token block i is emitted,
  then scores+exp+mask with PV trailing by 2 key tiles; the output
  projection for block i-1 is interleaved into block i so there is no
  serial phase C tail.
- Diagonal score/PV matmuls stream only the causally-live query suffix,
  and the mask multiply covers only the 128-wide triangle column block.
- Softmax normalization: denominators come from a ones column in V; a
  fast approximate reciprocal per pair is broadcast across partitions
  via a DRAM bounce, staggered two pairs behind the attention pipeline.
"""

import numpy as np
import ml_dtypes

import concourse.bacc as bacc
import concourse.tile as tile
import concourse.mybir as mybir
from concourse import bass_utils
from concourse.bass_interp import get_hw_module

bf16 = ml_dtypes.bfloat16
BF = mybir.dt.bfloat16
F32 = mybir.dt.float32
EXP = mybir.ActivationFunctionType.Exp

B, T, E = 4, 2048, 1024
H, HD = 16, 64
G = 2  # head groups (tensor-parallel dimension)
HL = H // G  # heads per core
DL = HL * HD  # 512 local feature dim
P = 128
NT = T // P  # 16 token tiles
NQ = T // 512  # 4 query blocks
EC = E // P  # 8 contraction chunks over E
DC = DL // P  # 4 chunks over local head dims (one per head pair)

_CACHE = {}
LAST_RESULT = None


def _build():
    nc = bacc.Bacc("TRN2", target_bir_lowering=False, debug=False, num_devices=8)
    xt_d = nc.dram_tensor("xt", (NT, P, EC, P), BF, kind="ExternalInput").ap()
    sin_d = nc.dram_tensor("sin", (P, 2, 4, 1024), BF, kind="ExternalInput").ap()
    cos_d = nc.dram_tensor("cos", (P, 2, 4, 1024), BF, kind="ExternalInput").ap()
    wq_d = nc.dram_tensor("wq", (P, EC, DL), BF, kind="ExternalInput").ap()
    wk_d = nc.dram_tensor("wk", (P, EC, DL), BF, kind="ExternalInput").ap()
    wv_d = nc.dram_tensor("wv", (P, EC, DL), BF, kind="ExternalInput").ap()
    wo_d = nc.dram_tensor("wo", (P, DC, E), BF, kind="ExternalInput").ap()
    mask_d = nc.dram_tensor("mask", (P, P), BF, kind="ExternalInput").ap()
    out_d = nc.dram_tensor("out", (T, E), BF, kind="ExternalOutput").ap()

    with tile.TileContext(nc) as tc:
        with (
            tc.tile_pool(name="persist", bufs=1) as persist,
            tc.tile_pool(name="tabs", bufs=1) as tabs,
            tc.tile_pool(name="tmps", bufs=1) as tmps,
            tc.tile_pool(name="att", bufs=6) as apool,
            tc.tile_pool(name="pks", bufs=2) as pkp,
            tc.tile_pool(name="rbs", bufs=3) as rbp,
            tc.tile_pool(name="ots", bufs=4) as otp,
            tc.tile_pool(name="dramn", bufs=3, space="DRAM") as dpool,
            tc.tile_pool(name="ps512", bufs=2, space="PSUM") as ps512,
            tc.tile_pool(name="sps", bufs=2, space="PSUM") as spool,
            tc.tile_pool(name="ops", bufs=2, space="PSUM") as opool,
        ):
            xt = persist.tile([P, NT, EC, P], BF)
            qT = persist.tile([P, DC, T], BF)
            kT = persist.tile([P, DC, T], BF)
            v = persist.tile([P, NT, HL, HD + 1], BF)
            oc = persist.tile([P, DC, T], BF)
            wq = persist.tile([P, EC, DL], BF)
            wk = persist.tile([P, EC, DL], BF)
            wv = persist.tile([P, EC, DL], BF)
            wo = persist.tile([P, DC, E], BF)
            mask = persist.tile([P, P], BF)
            sel = persist.tile([1, HD], BF)

            sins = [tabs.tile([P, 4, 1024], BF, name=f"s{h}") for h in range(2)]
            coss = [tabs.tile([P, 4, 1024], BF, name=f"c{h}") for h in range(2)]

            # ---- DMA emission = approximate arrival order; sequenced so
            # each consumer unblocks as early as possible: V-proj needs
            # wv+xt[k]; RoPE chunk 0 needs xt[0:4] + the per-u table slices;
            # the first K-proj needs wk + RoPE chunk 0.
            # Spread DMA issue over the two HWDGE queues (each issue costs
            # ~650ns serial on its queue; gpsimd/SWDGE generates descriptors
            # in ucode and is far too slow for bulk transfers).
            nc.sync.dma_start(wv[:, 0:2, :], wv_d[:, 0:2])
            nc.sync.dma_start(xt[:, 0:1], xt_d[0:1].rearrange("t p e c -> p t e c"))
            nc.sync.dma_start(wv[:, 2:4, :], wv_d[:, 2:4])
            nc.scalar.dma_start(sins[0][:, 2, :], sin_d[:, 0, 2])
            nc.scalar.dma_start(coss[0][:, 2, :], cos_d[:, 0, 2])
            nc.sync.dma_start(wv[:, 4:8, :], wv_d[:, 4:8])
            nc.scalar.dma_start(sins[0][:, 3, :], sin_d[:, 0, 3])
            nc.scalar.dma_start(coss[0][:, 3, :], cos_d[:, 0, 3])
            nc.sync.dma_start(xt[:, 1:2], xt_d[1:2].rearrange("t p e c -> p t e c"))
            nc.sync.dma_start(xt[:, 2:4], xt_d[2:4].rearrange("t p e c -> p t e c"))
            nc.sync.dma_start(sins[0][:, 0, :], sin_d[:, 0, 0])
            nc.sync.dma_start(coss[0][:, 0, :], cos_d[:, 0, 0])
            nc.scalar.dma_start(sins[0][:, 1, :], sin_d[:, 0, 1])
            nc.scalar.dma_start(coss[0][:, 1, :], cos_d[:, 0, 1])
            nc.scalar.dma_start(wq, wq_d)
            nc.sync.dma_start(xt[:, 4:6], xt_d[4:6].rearrange("t p e c -> p t e c"))
            nc.sync.dma_start(xt[:, 6:8], xt_d[6:8].rearrange("t p e c -> p t e c"))

            nc.vector.memset(v[:, :, :, HD : HD + 1], 1.0)
            nc.vector.memset(sel, 1.0)

            def emit_vproj(tk):
                vp = ps512.tile([P, DL], F32, tag="ps512")
                for j in range(EC):
                    nc.tensor.matmul(
                        vp,
                        lhsT=xt[:, tk, j, :],
                        rhs=wv[:, j, :],
                        start=(j == 0),
                        stop=(j == EC - 1),
                    )
                nc.scalar.copy(
                    v[:, tk, :, 0:HD], vp.rearrange("p (h d) -> p h d", h=HL)
                )

            def emit_rope(tc_):
                """RoPE (in place) for token chunk tc_ (512 tokens). All on
                DVE: offloading u-chunks to GpSimd was tested and reverted --
                concurrent engines on adjacent xt slices contend for SBUF
                ports and slow both to ~3x per-op time."""
                half, qh = tc_ // 2, tc_ % 2
                ts = slice(4 * tc_, 4 * tc_ + 4)
                cs = slice(512 * qh, 512 * qh + 512)
                for u in range(4):
                    # 4 DVE ops per u instead of 6: multiply the (even, odd)
                    # lane pair by cos and sin in one strided op each, then
                    # combine.  xp = xt[:, ts, u::4-stride over (u, u+4)]
                    xe = xt[:, ts, u, :]
                    xo = xt[:, ts, u + 4, :]
                    xp = xt[:, ts, u : u + 5 : 4, :]
                    s_u = sins[half][:, u, cs].rearrange("p (a b) -> p a b", a=4)
                    c_u = coss[half][:, u, cs].rearrange("p (a b) -> p a b", a=4)
                    s_b = s_u[:, :, None, :].to_broadcast((P, 4, 2, P))
                    c_b = c_u[:, :, None, :].to_broadcast((P, 4, 2, P))
                    tc2 = tmps.tile([P, 4, 2, P], BF, tag="tc2")
                    ts2 = tmps.tile([P, 4, 2, P], BF, tag="ts2")
                    nc.vector.tensor_mul(tc2, xp, c_b)
                    nc.vector.tensor_mul(ts2, xp, s_b)
                    nc.vector.tensor_sub(xe, tc2[:, :, 0, :], ts2[:, :, 1, :])
                    nc.vector.tensor_add(xo, tc2[:, :, 1, :], ts2[:, :, 0, :])

            def emit_qk(hp, i):
                """Q+K projection for token block i of pair hp."""
                tq = slice(512 * i, 512 * (i + 1))
                # Q first: scores consume fresh qT at j=0, but the fresh kT
                # tiles only at the diagonal (end of the j loop), so the Q
                # copy hides under the K projection matmuls.
                # contraction order follows RoPE completion (chunk pair
                # (u, u+4) is ready after RoPE u), so the projection starts
                # as soon as the first lane pair is rotated instead of
                # waiting for the whole chunk
                jorder = (0, 4, 1, 5, 2, 6, 3, 7)
                for w_sb, dst in ((wq, qT), (wk, kT)):
                    pp = ps512.tile([P, 512], F32, tag="ps512")
                    for n, j in enumerate(jorder):
                        nc.tensor.matmul(
                            pp,
                            lhsT=w_sb[:, j, P * hp : P * (hp + 1)],
                            rhs=xt[:, 4 * i : 4 * i + 4, j, :],
                            start=(n == 0),
                            stop=(n == EC - 1),
                        )
                    nc.vector.tensor_copy(dst[:, hp, tq], pp)

            def emit_outproj_groups(i, tsub, alt=False):
                """Output projection for token sub-tile tsub (0..3) of block i."""
                tt = 4 * i + tsub
                for et in range(2):
                    pp = ps512.tile([P, 512], F32, tag="ps512")
                    for kk in range(DC):
                        nc.tensor.matmul(
                            pp,
                            lhsT=oc[:, kk, P * tt : P * (tt + 1)],
                            rhs=wo[:, kk, 512 * et : 512 * (et + 1)],
                            start=(kk == 0),
                            stop=(kk == DC - 1),
                        )
                    ot = otp.tile([P, 512], BF, tag="ot")
                    if alt and et == 0:
                        nc.scalar.copy(ot, pp)
                    else:
                        nc.vector.tensor_copy(ot, pp)
                    eng = nc.scalar if (alt and et == 1) else nc.sync
                    eng.dma_start(
                        out_d[P * tt : P * (tt + 1), 512 * et : 512 * (et + 1)], ot
                    )

            def emit_attn(hp, i):
                """Scores+exp+mask with trailing PV for (block i, pair hp)."""
                h0, h1 = 2 * hp, 2 * hp + 1
                nj = 4 * i + 4
                tq0 = 512 * i
                op0 = opool.tile([HD + 1, 512], F32, tag="o")
                op1 = opool.tile([HD + 1, 512], F32, tag="o")
                ats = []

                def emit_pv(jp, at_jp):
                    lo = max(0, P * (jp - 4 * i))
                    nc.tensor.matmul(
                        op0[:, lo:512],
                        lhsT=v[:, jp, h0, :],
                        rhs=at_jp[:, 0, lo:512],
                        start=(jp == 0),
                        stop=(jp == nj - 1),
                    )
                    nc.tensor.matmul(
                        op1[:, lo:512],
                        lhsT=v[:, jp, h1, :],
                        rhs=at_jp[:, 1, lo:512],
                        start=(jp == 0),
                        stop=(jp == nj - 1),
                    )

                for j in range(nj):
                    r = j - 4 * i
                    lo = max(0, P * r)
                    sp = spool.tile([P, 2, 512], F32, tag="s")
                    nc.tensor.matmul(
                        sp[:, 0, lo:512],
                        lhsT=kT[0:HD, hp, P * j : P * (j + 1)],
                        rhs=qT[0:HD, hp, tq0 + lo : tq0 + 512],
                        start=True,
                        stop=True,
                    )
                    nc.tensor.matmul(
                        sp[:, 1, lo:512],
                        lhsT=kT[HD:P, hp, P * j : P * (j + 1)],
                        rhs=qT[HD:P, hp, tq0 + lo : tq0 + 512],
                        start=True,
                        stop=True,
                    )
                    at = apool.tile([P, 2, 512], BF, tag="a")
                    nc.scalar.activation(
                        at[:, :, lo:512], sp[:, :, lo:512], EXP, scale=0.125
                    )
                    if r >= 0:
                        # only the 128-wide diagonal block needs the triangle
                        # mask; columns beyond it are fully live
                        nc.vector.tensor_mul(
                            at[:, :, lo : lo + P],
                            at[:, :, lo : lo + P],
                            mask[:, None, :].to_broadcast((P, 2, P)),
                        )
                    ats.append(at)
                    if j >= 2:
                        emit_pv(j - 2, ats[j - 2])
                for jp in range(max(nj - 2, 0), nj):
                    emit_pv(jp, ats[jp])

                # denominators (ones-column rows) -> staging at partition 0
                # (engine APs need aligned partition bases)
                s0 = pkp.tile([1, 512], F32, tag="sd", bufs=4)
                s1 = pkp.tile([1, 512], F32, tag="sd", bufs=4)
                nc.scalar.copy(s0, op0[HD : HD + 1, :])
                nc.scalar.copy(s1, op1[HD : HD + 1, :])
                # oc copies are deferred into finish_norm / finish_norm_fast
                # (one pair later) so the next pair's fresh Q copy goes first
                # on the DVE queue
                return s0, s1, op0, op1

            def finish_norm(i, hp, s0, s1, op0, op1):
                """Normalize oc for (i, hp): reciprocal rows broadcast across
                partitions via two K=1 matmuls (ones(1,64) x row) into one
                PSUM tile, then a single fused multiply into oc. Emitted one
                pair later so the PE never waits on the reciprocal chain."""
                tq = slice(512 * i, 512 * (i + 1))
                nc.vector.tensor_copy(oc[0:HD, hp, tq], op0[0:HD, :])
                nc.vector.tensor_copy(oc[HD:P, hp, tq], op1[0:HD, :])
                r0 = pkp.tile([1, 512], F32, tag="re", bufs=2)
                r1 = pkp.tile([1, 512], F32, tag="re", bufs=2)
                nc.vector.reciprocal_approx_fast(r0, s0)
                nc.vector.reciprocal_approx_fast(r1, s1)
                rd = dpool.tile([2, 512], F32, tag="rd", name=f"rd{i}_{hp}")
                nc.sync.dma_start(rd[0:1, :], r0)
                nc.sync.dma_start(rd[1:2, :], r1)
                rb = rbp.tile([P, 512], F32, tag="rb")
                nc.sync.dma_start(rb[0:HD, :], rd[0:1, :].to_broadcast((HD, 512)))
                nc.sync.dma_start(rb[HD:P, :], rd[1:2, :].to_broadcast((HD, 512)))
                return rb

            def finish_mul(i, hp, rb):
                tq = slice(512 * i, 512 * (i + 1))
                nc.vector.tensor_mul(oc[:, hp, tq], oc[:, hp, tq], rb)

            def finish_norm_fast(i, hp, s0, s1, op0, op1):
                """Tail-only normalize: broadcast via two K=1 PE matmuls
                (ones(1,64) x reciprocal row) instead of the DMA bounce --
                the PE is idle at the tail, and this chain is ~4us shorter."""
                tq = slice(512 * i, 512 * (i + 1))
                r0 = pkp.tile([1, 512], F32, tag="re", bufs=2)
                r1 = pkp.tile([1, 512], F32, tag="re", bufs=2)
                nc.vector.reciprocal_approx_fast(r0, s0)
                nc.vector.reciprocal_approx_fast(r1, s1)
                rc0 = pkp.tile([1, 512], BF, tag="rc", bufs=2)
                rc1 = pkp.tile([1, 512], BF, tag="rc", bufs=2)
                nc.vector.tensor_copy(rc0, r0)
                nc.vector.tensor_copy(rc1, r1)
                nc.scalar.copy(oc[0:HD, hp, tq], op0[0:HD, :])
                nc.scalar.copy(oc[HD:P, hp, tq], op1[0:HD, :])
                rb = ps512.tile([P, 512], F32, tag="ps512")
                nc.tensor.matmul(rb[0:HD, :], lhsT=sel, rhs=rc0, start=True, stop=True)
                nc.tensor.matmul(rb[HD:P, :], lhsT=sel, rhs=rc1, start=True, stop=True)
                nc.vector.tensor_mul(oc[:, hp, tq], oc[:, hp, tq], rb)

            # ---- pipeline ----
            for tk in range(6):
                emit_vproj(tk)
            nc.scalar.dma_start(mask, mask_d)
            emit_vproj(6)
            emit_vproj(7)
            # late-needed bytes (~6MB): issued from the scalar queue behind
            # the V-proj copies so their descriptors don't steal bandwidth
            # from the startup-critical set
            nc.scalar.dma_start(wk, wk_d)
            nc.scalar.dma_start(xt[:, 8:12], xt_d[8:12].rearrange("t p e c -> p t e c"))
            nc.scalar.dma_start(xt[:, 12:16], xt_d[12:16].rearrange("t p e c -> p t e c"))
            nc.scalar.dma_start(sins[1], sin_d[:, 1])
            nc.scalar.dma_start(coss[1], cos_d[:, 1])
            nc.scalar.dma_start(wo, wo_d)
            emit_rope(0)

            bounce_q = None  # (i, hp, s0, s1): reciprocal+broadcast next pair
            mul_q = None  # (i, hp, rb): normalize-multiply the pair after
            for i in range(NQ):
                for hp in range(DC):
                    emit_qk(hp, i)
                    if mul_q is not None:
                        finish_mul(*mul_q)
                        mul_q = None
                    if bounce_q is not None:
                        bi, bhp = bounce_q[0], bounce_q[1]
                        mul_q = (bi, bhp, finish_norm(*bounce_q))
                    bounce_q = (i, hp) + emit_attn(hp, i)
                    if i == 0:
                        emit_vproj(8 + 2 * hp)
                        emit_vproj(9 + 2 * hp)
                    if hp == 2 and i < 3:
                        emit_rope(i + 1)
                    if i > 0 and hp >= 1:
                        emit_outproj_groups(i - 1, hp - 1)
                        if hp == 3:
                            emit_outproj_groups(i - 1, 3)
            bi, bhp = bounce_q[0], bounce_q[1]
            finish_mul(*mul_q)
            finish_norm_fast(bi, bhp, *bounce_q[2:])
            for tsub in range(3):
                emit_outproj_groups(NQ - 1, tsub, alt=True)
            # last token tile: 256-wide units so the final copy->DMA chain
            # drains at half granularity
            tt = 4 * (NQ - 1) + 3
            for eq in range(4):
                pp = ps512.tile([P, 256], F32, tag="ps512")
                for kk in range(DC):
                    nc.tensor.matmul(
                        pp,
                        lhsT=oc[:, kk, P * tt : P * (tt + 1)],
                        rhs=wo[:, kk, 256 * eq : 256 * (eq + 1)],
                        start=(kk == 0),
                        stop=(kk == DC - 1),
                    )
                ot = otp.tile([P, 256], BF, tag="ot")
                if eq % 2 == 0:
                    nc.scalar.copy(ot, pp)
                else:
                    nc.vector.tensor_copy(ot, pp)
                eng = nc.scalar if eq % 2 else nc.sync
                eng.dma_start(
                    out_d[P * tt : P * (tt + 1), 256 * eq : 256 * (eq + 1)], ot
                )

    nc.compile()
    nc.m = get_hw_module(nc.m)
    return nc


def _prep_inputs(input, Wq, Wk, Wv, Wo):
    """Host-side shard prep: transpose/de-interleave/cast. Returns 8 in_maps."""
    perm = np.concatenate([np.arange(0, E, 2), np.arange(1, E, 2)])

    u = np.arange(E // 2, dtype=np.float64)
    thetas = 10000.0 ** (-2.0 * u / E)
    ang = np.arange(T, dtype=np.float64)[:, None] * thetas[None, :]
    # sin_h[p, half, u, t'] = sin(ang[1024*half + t', u*128 + p])
    sin_h = np.ascontiguousarray(
        np.sin(ang).T.reshape(4, P, 2, 1024).transpose(1, 2, 0, 3)
    ).astype(bf16)
    cos_h = np.ascontiguousarray(
        np.cos(ang).T.reshape(4, P, 2, 1024).transpose(1, 2, 0, 3)
    ).astype(bf16)

    f = np.arange(P)
    mask = (f[None, :] >= f[:, None]).astype(np.float32).astype(bf16)

    xt = []
    for b in range(B):
        xc = input[b].T[perm].reshape(EC, P, NT, P)
        xt.append(np.ascontiguousarray(xc.transpose(2, 1, 0, 3)).astype(bf16))
    WqT, WkT, WvT = Wq.T[perm], Wk.T[perm], Wv.T[perm]

    def wslice(WT, g):
        w = WT[:, DL * g : DL * (g + 1)].reshape(EC, P, DL)
        return np.ascontiguousarray(w.transpose(1, 0, 2)).astype(bf16)

    wq_g = [wslice(WqT, g) for g in range(G)]
    wk_g = [wslice(WkT, g) for g in range(G)]
    wv_g = [wslice(WvT, g) for g in range(G)]
    wo_g = [
        np.ascontiguousarray(
            Wo.T[DL * g : DL * (g + 1)].reshape(DC, P, E).transpose(1, 0, 2)
        ).astype(bf16)
        for g in range(G)
    ]

    in_maps = []
    for c in range(8):
        b, g = c // 2, c % 2
        in_maps.append(
            {
                "xt": xt[b],
                "sin": sin_h,
                "cos": cos_h,
                "wq": wq_g[g],
                "wk": wk_g[g],
                "wv": wv_g[g],
                "wo": wo_g[g],
                "mask": mask,
            }
        )
    return in_maps


def kernel(input, Wq, Wk, Wv, Wo, bo):
    global LAST_RESULT
    input = np.asarray(input, np.float32)
    Wq, Wk, Wv, Wo = (np.asarray(w, np.float32) for w in (Wq, Wk, Wv, Wo))
    bo = np.asarray(bo, np.float32)

    if "nc" not in _CACHE:
        _CACHE["nc"] = _build()
    nc = _CACHE["nc"]

    in_maps = _prep_inputs(input, Wq, Wk, Wv, Wo)
    res = bass_utils.run_bass_kernel_spmd(nc, in_maps, core_ids=list(range(8)))
    LAST_RESULT = res

    out = np.empty((B, T, E), np.float32)
    for b in range(B):
        out[b] = (
            res.results[2 * b]["out"].astype(np.float32)
            + res.results[2 * b + 1]["out"].astype(np.float32)
            + bo
        )
    return out
